# revision 37
# baseline (speedup 1.0000x reference)
"""MHSA over 32 independent 512-token segments, segment-parallel across 8
NeuronCores (4 segments / 2048 tokens per core, zero cross-core traffic).

All operands bf16 (converted host-side); matmul accumulation f32 in PSUM.
w_attn / w_proj are SBUF-resident for the whole kernel (loaded once).
Both transposes (x^T on load, Y^T before projection) run on the DMA xbar,
one whole-segment DMA each - the PE does only matmuls.

Per core, per segment s:
  x^T        via DMA transpose (HBM -> SBUF)              [128, 8c x 512]
  Q^T,K^T    = (W^T x^T) : lhsT=W chunks, rhs=x^T         16x [128, 512]
  V          = x @ Wv    : lhsT=x^T chunks, rhs=W         natural [tok, 1024]
  S^T        = K Q^T per head (K=64)                      [512k, 512q] psum
  A^T        = exp(S^T/8)  bf16 (no max-sub: |S/8|<~6)    [512, 512]
  O          = A^T.T @ [V|1] per q-chunk: out [128q, 65]  (col 64 = rowsum Z)
  Y          = O[:, 0:64] * (1/Z)  per-partition scalar   (DVE, no PE)
  Y^T        via DMA transpose (SBUF -> SBUF)             [128, 32b x 128]
  out        = Y^T.T @ Wproj

Schedule: phase C (attention) of segment s is interleaved with the QKV
matmuls of segment s+1 (and, for the last segment, with the previous
segment's projection) so the PE never waits on the Act-bound exp chain.

PSUM: 4 separate S^T tiles (banks 0-3, 4-deep rotation against exp),
2 A.V tiles (4-5), 2 QKV accumulation tiles (6-7). Separate tiles matter:
sync tracking is tile-granular, a shared tile serializes rotation.
"""

import numpy as np
import ml_dtypes

import concourse.bass as bass
import concourse.mybir as mybir
import concourse.tile as tile
from concourse.bass_utils import run_bass_kernel_spmd

F32 = mybir.dt.float32
BF16 = mybir.dt.bfloat16
EXP = mybir.ActivationFunctionType.Exp

T, C, H, HD = 16384, 1024, 16, 64
NCORES = 8
TOK = T // NCORES          # 2048 tokens per core
SEG = 512                  # tokens per segment
NSEG = TOK // SEG          # 4 segments per core
SCALE = 1.0 / np.sqrt(HD)  # folded into exp()

COST_B, COST_D = 1710, 1750
SLOT_FILL = 1050           # target filler ns per head slot


def _split_multi_waits(nc):
    """Move extra sync waits onto same-engine NoOps (1-wait ISA limit).
    Same-engine waits go on the NoOps: buffer rotation is deep enough that
    they reference old, already-fired semaphores, while the cross-engine
    wait stays on the instruction so the sequencer can queue ahead."""
    for fn in nc.m.functions:
        for bb in fn.blocks:
            out = []
            for inst in bb.instructions:
                si = inst.sync_info
                eng = str(inst.engine).split(".")[-1]
                if si is not None and si.on_wait and len(si.on_wait) > 1:
                    waits = sorted(
                        si.on_wait,
                        key=lambda w: not (
                            isinstance(w.ant_name, str)
                            and w.ant_name.split("_")[0] == eng
                        ),
                    )
                    for j, w in enumerate(waits[:-1]):
                        nop = mybir.InstNoOp(name=f"{inst.name}-wsp{j}")
                        nop.engine = inst.engine
                        nop.sync_info = mybir.SyncInfo(on_wait=[w], on_update=[])
                        out.append(nop)
                    inst.sync_info = mybir.SyncInfo(
                        on_wait=[waits[-1]], on_update=list(si.on_update)
                    )
                out.append(inst)
            bb.instructions = out


def _build():
    nc = bass.Bass("TRN2", target_bir_lowering=False, debug=False)
    x = nc.dram_tensor("x_sh", [TOK, C], BF16, kind="ExternalInput").ap()
    wa = nc.dram_tensor("w_attn", [C, 3 * C], BF16, kind="ExternalInput").ap()
    wp = nc.dram_tensor("w_proj", [C, C], BF16, kind="ExternalInput").ap()
    out = nc.dram_tensor("out", [TOK, C], F32, kind="ExternalOutput").ap()

    with tile.TileContext(nc) as tc:
        with (
            tc.tile_pool(name="wres", bufs=1) as wres,
            tc.tile_pool(name="work", bufs=1) as work,
            tc.tile_pool(name="ps", bufs=1, space="PSUM") as pspool,
        ):
            # ---- PSUM: separate tiles (see module docstring); S^T staging
            # as two 2-bank tiles so exp covers a pair in one instruction
            sb = [pspool.tile([128, 1024], F32, tag=f"sb{i}", name=f"sb{i}")
                  for i in range(2)]                                     # banks 0-3
            av = [pspool.tile([128, 512], F32, tag=f"av{i}", name=f"av{i}")
                  for i in range(2)]                                     # banks 4-5
            bu = [pspool.tile([128, 512], F32, tag=f"bu{i}", name=f"bu{i}")
                  for i in range(2)]                                     # banks 6-7

            def sbank(i):
                return sb[i // 2][:, (i % 2) * 512:(i % 2) * 512 + 512]

            # ---- resident weights (bf16, loaded once)
            wa_sb = [wres.tile([128, 3 * C], BF16, tag=f"wa{cc}", name=f"wa{cc}")
                     for cc in range(8)]
            wp_sb = [wres.tile([128, C], BF16, tag=f"wp{cc}", name=f"wp{cc}")
                     for cc in range(8)]

            # ---- per-segment working tiles, double-buffered by parity
            # x^T: channel-chunk blocks of 512 tokens: [128, cc(8) x 512]
            xT = [work.tile([128, 4096], BF16, tag=f"xT{p}", name=f"xT{p}")
                  for p in range(2)]
            qkt = [[work.tile([128, SEG], BF16, tag=f"qkt{p}_{m}", name=f"qkt{p}_{m}")
                    for m in range(16)] for p in range(2)]
            vp = [[work.tile([128, 16 * 66], BF16, tag=f"vp{p}_{qt}", name=f"vp{p}_{qt}")
                   for qt in range(4)] for p in range(2)]
            # Y (pre-transpose), per q-chunk: [128, 16h x 64]
            ytp = [[work.tile([128, 1024], BF16, tag=f"ytp{p}_{qc}",
                              name=f"ytp{p}_{qc}") for qc in range(4)]
                   for p in range(2)]
            # Y^T: (qc x cc)-blocks of 128 tokens: [128, 32b x 128]
            ytr = work.tile([128, 4096], BF16, tag="ytr", name="ytr")

            # ones columns of vp (col 64 of each 66-stride head block) persist
            for p in range(2):
                for qt in range(4):
                    nc.vector.memset(
                        vp[p][qt].rearrange("p (h w) -> p h w", w=66)[:, :, 64:65], 1.0
                    )

            # ---------- emission helpers ----------
            def dma_xT(s):
                # whole-segment x^T straight off HBM via the DMA xbar
                nc.sync.dma_start_transpose(
                    xT[s % 2].rearrange("p (c t) -> p c t", t=512),
                    x[s * SEG:(s + 1) * SEG, :],
                )

            def qk_unit(s, u, b):
                # Q^T/K^T channel chunk: g = u//8 (0=Q,1=K), m = u%8
                p = s % 2
                g, m = u // 8, u % 8
                for cc in range(8):
                    nc.tensor.matmul(
                        b[:, :],
                        wa_sb[cc][:, g * C + m * 128: g * C + (m + 1) * 128],
                        xT[p][:, cc * 512:(cc + 1) * 512],
                        start=(cc == 0), stop=(cc == 7),
                    )
                nc.vector.tensor_copy(qkt[p][g * 8 + m][:], b[:, :])

            def v_unit(s, u, b):
                # V tok-chunk qt = u//2, channel half vn = u%2
                p = s % 2
                qt, vn = u // 2, u % 2
                for cc in range(8):
                    nc.tensor.matmul(
                        b[:, :],
                        xT[p][:, cc * 512 + qt * 128: cc * 512 + (qt + 1) * 128],
                        wa_sb[cc][:, 2 * C + vn * 512: 2 * C + (vn + 1) * 512],
                        start=(cc == 0), stop=(cc == 7),
                    )
                nc.vector.tensor_copy(
                    vp[p][qt].rearrange("p (h w) -> p h w", w=66)[:, vn * 8:(vn + 1) * 8, 0:64],
                    b.rearrange("p (h w) -> p h w", w=64),
                )

            at0s = {}

            def head_s(s, h):
                # S^T chunks (4-deep bank rotation) + per-chunk exp -> at0
                p = s % 2
                qk_q = qkt[p][h // 2]
                qk_k = qkt[p][8 + h // 2]
                r0 = (h % 2) * 64
                at0 = at0s[h % 3] = work.tile([128, 2048], BF16, tag="at0",
                                              bufs=3, name=f"at0_{s}_{h}")
                for kt in range(4):
                    nc.tensor.matmul(
                        sbank(kt), qk_k[r0:r0 + 64, kt * 128:(kt + 1) * 128],
                        qk_q[r0:r0 + 64, :], start=True, stop=True,
                    )
                    if kt % 2 == 1:
                        nc.scalar.activation(
                            at0[:, (kt - 1) * 512:(kt + 1) * 512],
                            sb[kt // 2][:, :], EXP, scale=SCALE,
                        )

            def head_av(s, h):
                p = s % 2
                at0 = at0s[h % 3]
                a = av[h % 2]
                for qc in range(4):
                    for kt in range(4):
                        nc.tensor.matmul(
                            a[:, qc * 128: qc * 128 + 65],
                            at0[:, kt * 512 + qc * 128: kt * 512 + (qc + 1) * 128],
                            vp[p][kt][:, 66 * h: 66 * h + 65],
                            start=(kt == 0), stop=(kt == 3),
                        )
                zrec = work.tile([128, 4], F32, tag="zrec", bufs=4,
                                 name=f"zrec_{s}_{h}")
                nc.vector.reciprocal(
                    zrec[:, :],
                    a.rearrange("p (q w) -> p q w", w=128)[:, :, 64:65],
                )
                for qc in range(4):
                    nc.vector.tensor_scalar_mul(
                        ytp[p][qc][:, 64 * h: 64 * h + 64],
                        a[:, qc * 128: qc * 128 + 64],
                        zrec[:, qc:qc + 1],
                    )

            def dma_ytr(s):
                # Y^T via per-q-chunk SBUF->SBUF DMA transposes; each waits
                # only its own chunk's divides and pipelines with the rest
                ytv = ytr.rearrange("p (c t) -> p c t", t=128)
                for qc in range(4):
                    nc.scalar.dma_start_transpose(
                        ytv[:, qc * 8:(qc + 1) * 8, :], ytp[s % 2][qc][:, :]
                    )

            deferred_dmas = []

            def proj_half(s, m, vn, half, b, ob, eng, defer=False):
                for j in range(4):
                    cc = half * 4 + j
                    nc.tensor.matmul(
                        b[:, :],
                        ytr[:, (m * 8 + cc) * 128:(m * 8 + cc + 1) * 128],
                        wp_sb[cc][:, vn * 512:(vn + 1) * 512],
                        start=(cc == 0), stop=(cc == 7),
                    )
                if half == 1:
                    eng(ob[:, vn * 512:(vn + 1) * 512], b[:, :])
                    rows = out[s * SEG + m * 128: s * SEG + (m + 1) * 128, :]
                    if s == NSEG - 1 and m == 3:
                        nc.sync.dma_start(
                            rows[:, vn * 512:(vn + 1) * 512],
                            ob[:, vn * 512:(vn + 1) * 512],
                        )
                    elif vn == 1:
                        if defer:
                            deferred_dmas.append(
                                lambda rows=rows, ob=ob: nc.sync.dma_start(rows, ob[:]))
                        else:
                            nc.sync.dma_start(rows, ob[:])

            def d_units(s, pbanks, eng, split):
                # projection units for segment s (dma_ytr(s) must precede)
                units = []
                obs = {}

                def mk_ob(m):
                    obs[m] = work.tile([128, C], F32, tag="ob", bufs=4,
                                       name=f"ob{s}_{m}")
                    return obs[m]

                for m in range(4):
                    for vn in range(2):
                        b = pbanks[(m * 2 + vn) % len(pbanks)]
                        ob = mk_ob(m) if vn == 0 else obs[m]
                        if split:
                            for half in range(2):
                                units.append((COST_D // 2, lambda s=s, m=m, vn=vn,
                                              half=half, b=b, ob=ob: proj_half(
                                                  s, m, vn, half, b, ob, eng,
                                                  defer=True)))
                        else:
                            units.append((COST_D, lambda s=s, m=m, vn=vn, b=b,
                                          ob=ob: (proj_half(s, m, vn, 0, b, ob, eng),
                                                  proj_half(s, m, vn, 1, b, ob, eng))))
                return units

            # ---------- program ----------
            # prologue DMA order (DMA issue/transfer are serialized resources):
            # x^T(seg0) -> Wq -> Wk -> x^T(seg1) -> Wv -> Wproj
            dma_xT(0)
            for cc in range(8):
                nc.scalar.dma_start(wa_sb[cc][:, 0:128],
                                    wa[cc * 128:(cc + 1) * 128, 0:128])
            for cc in range(8):
                nc.scalar.dma_start(wa_sb[cc][:, 128:C],
                                    wa[cc * 128:(cc + 1) * 128, 128:C])
            for cc in range(8):
                nc.sync.dma_start(wa_sb[cc][:, C:2 * C],
                                  wa[cc * 128:(cc + 1) * 128, C:2 * C])
            dma_xT(1)
            for cc in range(8):
                nc.sync.dma_start(wa_sb[cc][:, 2 * C:3 * C],
                                  wa[cc * 128:(cc + 1) * 128, 2 * C:3 * C])
            for cc in range(8):
                nc.sync.dma_start(wp_sb[cc][:], wp[cc * 128:(cc + 1) * 128, :])
            for u in range(16):
                qk_unit(0, u, bu[u % 2])
            for u in range(8):
                v_unit(0, u, bu[u % 2])

            for s in range(NSEG):
                if s + 2 < NSEG:
                    dma_xT(s + 2)
                if s + 1 < NSEG:
                    fillers = [(COST_B, lambda b, s=s, u=u: qk_unit(s + 1, u, b))
                               for u in range(16)]
                    fillers += [(COST_B, lambda b, s=s, u=u: v_unit(s + 1, u, b))
                                for u in range(8)]
                    dpaced = False
                else:
                    fillers = [(c, lambda b, f=f: f())
                               for c, f in d_units(s - 1, [bu[0], bu[1]],
                                                   nc.vector.tensor_copy, True)]
                    dpaced = True  # one unit per slot: 16 units / 16 slots
                nb = 0
                for h in range(H):
                    head_s(s, h)
                    if h > 0:
                        head_av(s, h - 1)
                    got = big = n = 0
                    while fillers and got < SLOT_FILL and big < 1 \
                            and not (dpaced and n >= 1):
                        c, f = fillers.pop(0)
                        f(bu[nb % 2])
                        nb += 1
                        got += c
                        big += c > 500
                        n += 1
                if fillers:
                    _, f = fillers.pop(0)
                    f(bu[nb % 2])
                    nb += 1
                head_av(s, H - 1)
                dma_ytr(s)  # issue early: latency hides under the drain
                for dd in deferred_dmas:
                    dd()
                deferred_dmas.clear()
                for _, f in fillers:
                    f(bu[nb % 2])
                    nb += 1
                if s + 1 < NSEG:
                    if s != NSEG - 2:
                        for _, f in d_units(s, [sbank(0), sbank(1), sbank(2),
                                               sbank(3)], nc.scalar.copy, False):
                            f()
                else:
                    for _, f in d_units(s, [sbank(0), sbank(1), sbank(2),
                                           sbank(3)], nc.scalar.copy, False):
                        f()

    _split_multi_waits(nc)
    return nc


_NC = None


def kernel(x, w_attn, w_proj, split_sections):
    global _NC
    if _NC is None:
        _NC = _build()
    x = np.asarray(x, dtype=np.float32).astype(ml_dtypes.bfloat16)
    w_attn = np.asarray(w_attn, dtype=np.float32).astype(ml_dtypes.bfloat16)
    w_proj = np.asarray(w_proj, dtype=np.float32).astype(ml_dtypes.bfloat16)
    in_maps = [
        {"x_sh": np.ascontiguousarray(x[i * TOK:(i + 1) * TOK]),
         "w_attn": w_attn, "w_proj": w_proj}
        for i in range(NCORES)
    ]
    res = run_bass_kernel_spmd(_NC, in_maps, core_ids=list(range(NCORES)))
    return np.concatenate([res.results[i]["out"] for i in range(NCORES)], axis=0)


if __name__ == "__main__":
    rng = np.random.default_rng(0)
    x = rng.standard_normal((T, C), dtype=np.float32)
    wa = (rng.standard_normal((C, 3 * C), dtype=np.float32) / np.sqrt(C)).astype(np.float32)
    wpj = (rng.standard_normal((C, C), dtype=np.float32) / np.sqrt(C)).astype(np.float32)
    y = kernel(x, wa, wpj, np.arange(1, 32) * 512)
    print("out", y.shape, y.dtype, np.abs(y).mean())


# revision 38
# speedup vs baseline: 1.0230x; 1.0230x over previous
"""MHSA over 32 independent 512-token segments, segment-parallel across 8
NeuronCores (4 segments / 2048 tokens per core, zero cross-core traffic).

All operands bf16 (converted host-side); matmul accumulation f32 in PSUM.
w_attn / w_proj are SBUF-resident for the whole kernel (loaded once).
Both transposes (x^T on load, Y^T before projection) run on the DMA xbar,
one whole-segment DMA each - the PE does only matmuls.

Per core, per segment s:
  x^T        via DMA transpose (HBM -> SBUF)              [128, 8c x 512]
  Q^T,K^T    = (W^T x^T) : lhsT=W chunks, rhs=x^T         16x [128, 512]
  V          = x @ Wv    : lhsT=x^T chunks, rhs=W         natural [tok, 1024]
  S^T        = K Q^T per head (K=64)                      [512k, 512q] psum
  A^T        = exp(S^T/8)  bf16 (no max-sub: |S/8|<~6)    [512, 512]
  O          = A^T.T @ [V|1] per q-chunk: out [128q, 65]  (col 64 = rowsum Z)
  Y          = O[:, 0:64] * (1/Z)  per-partition scalar   (DVE, no PE)
  Y^T        via DMA transpose (SBUF -> SBUF)             [128, 32b x 128]
  out        = Y^T.T @ Wproj

Schedule: phase C (attention) of segment s is interleaved with the QKV
matmuls of segment s+1 (and, for the last segment, with the previous
segment's projection) so the PE never waits on the Act-bound exp chain.

PSUM: 4 separate S^T tiles (banks 0-3, 4-deep rotation against exp),
2 A.V tiles (4-5), 2 QKV accumulation tiles (6-7). Separate tiles matter:
sync tracking is tile-granular, a shared tile serializes rotation.
"""

import numpy as np
import ml_dtypes

import concourse.bass as bass
import concourse.mybir as mybir
import concourse.tile as tile
from concourse.bass_utils import run_bass_kernel_spmd

F32 = mybir.dt.float32
BF16 = mybir.dt.bfloat16
EXP = mybir.ActivationFunctionType.Exp

T, C, H, HD = 16384, 1024, 16, 64
NCORES = 8
TOK = T // NCORES          # 2048 tokens per core
SEG = 512                  # tokens per segment
NSEG = TOK // SEG          # 4 segments per core
SCALE = 1.0 / np.sqrt(HD)  # folded into exp()

COST_B, COST_D = 1710, 1750
SLOT_FILL = 1050           # target filler ns per head slot


def _split_multi_waits(nc):
    """Move extra sync waits onto same-engine NoOps (1-wait ISA limit).
    Same-engine waits go on the NoOps: buffer rotation is deep enough that
    they reference old, already-fired semaphores, while the cross-engine
    wait stays on the instruction so the sequencer can queue ahead."""
    for fn in nc.m.functions:
        for bb in fn.blocks:
            out = []
            for inst in bb.instructions:
                si = inst.sync_info
                eng = str(inst.engine).split(".")[-1]
                if si is not None and si.on_wait and len(si.on_wait) > 1:
                    waits = sorted(
                        si.on_wait,
                        key=lambda w: not (
                            isinstance(w.ant_name, str)
                            and w.ant_name.split("_")[0] == eng
                        ),
                    )
                    for j, w in enumerate(waits[:-1]):
                        nop = mybir.InstNoOp(name=f"{inst.name}-wsp{j}")
                        nop.engine = inst.engine
                        nop.sync_info = mybir.SyncInfo(on_wait=[w], on_update=[])
                        out.append(nop)
                    inst.sync_info = mybir.SyncInfo(
                        on_wait=[waits[-1]], on_update=list(si.on_update)
                    )
                out.append(inst)
            bb.instructions = out


def _build():
    nc = bass.Bass("TRN2", target_bir_lowering=False, debug=False)
    x = nc.dram_tensor("x_sh", [TOK, C], BF16, kind="ExternalInput").ap()
    wa = nc.dram_tensor("w_attn", [C, 3 * C], BF16, kind="ExternalInput").ap()
    wp = nc.dram_tensor("w_proj", [C, C], BF16, kind="ExternalInput").ap()
    out = nc.dram_tensor("out", [TOK, C], F32, kind="ExternalOutput").ap()

    with tile.TileContext(nc) as tc:
        with (
            tc.tile_pool(name="wres", bufs=1) as wres,
            tc.tile_pool(name="work", bufs=1) as work,
            tc.tile_pool(name="ps", bufs=1, space="PSUM") as pspool,
        ):
            # ---- PSUM: separate tiles (see module docstring); S^T staging
            # as two 2-bank tiles so exp covers a pair in one instruction
            sb = [pspool.tile([128, 1024], F32, tag=f"sb{i}", name=f"sb{i}")
                  for i in range(2)]                                     # banks 0-3
            av = [pspool.tile([128, 512], F32, tag=f"av{i}", name=f"av{i}")
                  for i in range(2)]                                     # banks 4-5
            bu = [pspool.tile([128, 512], F32, tag=f"bu{i}", name=f"bu{i}")
                  for i in range(2)]                                     # banks 6-7

            def sbank(i):
                return sb[i // 2][:, (i % 2) * 512:(i % 2) * 512 + 512]

            # ---- resident weights (bf16, loaded once)
            wa_sb = [wres.tile([128, 3 * C], BF16, tag=f"wa{cc}", name=f"wa{cc}")
                     for cc in range(8)]
            wp_sb = [wres.tile([128, C], BF16, tag=f"wp{cc}", name=f"wp{cc}")
                     for cc in range(8)]

            # ---- per-segment working tiles, double-buffered by parity
            # x^T: channel-chunk blocks of 512 tokens: [128, cc(8) x 512]
            xT = [work.tile([128, 4096], BF16, tag=f"xT{p}", name=f"xT{p}")
                  for p in range(2)]
            qkt = [[work.tile([128, SEG], BF16, tag=f"qkt{p}_{m}", name=f"qkt{p}_{m}")
                    for m in range(16)] for p in range(2)]
            vp = [[work.tile([128, 16 * 66], BF16, tag=f"vp{p}_{qt}", name=f"vp{p}_{qt}")
                   for qt in range(4)] for p in range(2)]
            # Y (pre-transpose), per q-chunk: [128, 16h x 64]
            ytp = [[work.tile([128, 1024], BF16, tag=f"ytp{p}_{qc}",
                              name=f"ytp{p}_{qc}") for qc in range(4)]
                   for p in range(2)]
            # Y^T: (qc x cc)-blocks of 128 tokens: [128, 32b x 128]
            ytr = work.tile([128, 4096], BF16, tag="ytr", name="ytr")

            # ones columns of vp (col 64 of each 66-stride head block) persist
            for p in range(2):
                for qt in range(4):
                    nc.vector.memset(
                        vp[p][qt].rearrange("p (h w) -> p h w", w=66)[:, :, 64:65], 1.0
                    )

            # ---------- emission helpers ----------
            def dma_xT(s):
                # whole-segment x^T straight off HBM via the DMA xbar
                nc.sync.dma_start_transpose(
                    xT[s % 2].rearrange("p (c t) -> p c t", t=512),
                    x[s * SEG:(s + 1) * SEG, :],
                )

            def qk_unit(s, u, b):
                # Q^T/K^T channel chunk: g = u//8 (0=Q,1=K), m = u%8
                p = s % 2
                g, m = u // 8, u % 8
                for cc in range(8):
                    nc.tensor.matmul(
                        b[:, :],
                        wa_sb[cc][:, g * C + m * 128: g * C + (m + 1) * 128],
                        xT[p][:, cc * 512:(cc + 1) * 512],
                        start=(cc == 0), stop=(cc == 7),
                    )
                nc.vector.tensor_copy(qkt[p][g * 8 + m][:], b[:, :])

            def v_unit(s, u, b):
                # V tok-chunk qt = u//2, channel half vn = u%2
                p = s % 2
                qt, vn = u // 2, u % 2
                for cc in range(8):
                    nc.tensor.matmul(
                        b[:, :],
                        xT[p][:, cc * 512 + qt * 128: cc * 512 + (qt + 1) * 128],
                        wa_sb[cc][:, 2 * C + vn * 512: 2 * C + (vn + 1) * 512],
                        start=(cc == 0), stop=(cc == 7),
                    )
                nc.vector.tensor_copy(
                    vp[p][qt].rearrange("p (h w) -> p h w", w=66)[:, vn * 8:(vn + 1) * 8, 0:64],
                    b.rearrange("p (h w) -> p h w", w=64),
                )

            at0s = {}

            def head_s(s, h):
                # S^T chunks (4-deep bank rotation) + per-chunk exp -> at0
                p = s % 2
                qk_q = qkt[p][h // 2]
                qk_k = qkt[p][8 + h // 2]
                r0 = (h % 2) * 64
                at0 = at0s[h % 3] = work.tile([128, 2048], BF16, tag="at0",
                                              bufs=3, name=f"at0_{s}_{h}")
                for kt in range(4):
                    nc.tensor.matmul(
                        sbank(kt), qk_k[r0:r0 + 64, kt * 128:(kt + 1) * 128],
                        qk_q[r0:r0 + 64, :], start=True, stop=True,
                    )
                    if kt % 2 == 1:
                        nc.scalar.activation(
                            at0[:, (kt - 1) * 512:(kt + 1) * 512],
                            sb[kt // 2][:, :], EXP, scale=SCALE,
                        )

            def head_av(s, h):
                p = s % 2
                at0 = at0s[h % 3]
                a = av[h % 2]
                for qc in range(4):
                    for kt in range(4):
                        nc.tensor.matmul(
                            a[:, qc * 128: qc * 128 + 65],
                            at0[:, kt * 512 + qc * 128: kt * 512 + (qc + 1) * 128],
                            vp[p][kt][:, 66 * h: 66 * h + 65],
                            start=(kt == 0), stop=(kt == 3),
                        )
                zrec = work.tile([128, 4], F32, tag="zrec", bufs=4,
                                 name=f"zrec_{s}_{h}")
                nc.vector.reciprocal(
                    zrec[:, :],
                    a.rearrange("p (q w) -> p q w", w=128)[:, :, 64:65],
                )
                for qc in range(4):
                    nc.vector.tensor_scalar_mul(
                        ytp[p][qc][:, 64 * h: 64 * h + 64],
                        a[:, qc * 128: qc * 128 + 64],
                        zrec[:, qc:qc + 1],
                    )

            def dma_ytr(s):
                # Y^T via per-q-chunk SBUF->SBUF DMA transposes; each waits
                # only its own chunk's divides and pipelines with the rest
                ytv = ytr.rearrange("p (c t) -> p c t", t=128)
                for qc in range(4):
                    nc.scalar.dma_start_transpose(
                        ytv[:, qc * 8:(qc + 1) * 8, :], ytp[s % 2][qc][:, :]
                    )

            deferred_dmas = []

            def proj_half(s, m, vn, half, b, ob, eng, defer=False):
                for j in range(4):
                    cc = half * 4 + j
                    nc.tensor.matmul(
                        b[:, :],
                        ytr[:, (m * 8 + cc) * 128:(m * 8 + cc + 1) * 128],
                        wp_sb[cc][:, vn * 512:(vn + 1) * 512],
                        start=(cc == 0), stop=(cc == 7),
                    )
                if half == 1:
                    eng(ob[:, vn * 512:(vn + 1) * 512], b[:, :])
                    rows = out[s * SEG + m * 128: s * SEG + (m + 1) * 128, :]
                    if s == NSEG - 1 and m == 3:
                        nc.sync.dma_start(
                            rows[:, vn * 512:(vn + 1) * 512],
                            ob[:, vn * 512:(vn + 1) * 512],
                        )
                    elif vn == 1:
                        if defer:
                            deferred_dmas.append(
                                lambda rows=rows, ob=ob: nc.sync.dma_start(rows, ob[:]))
                        else:
                            nc.sync.dma_start(rows, ob[:])

            def d_units(s, pbanks, eng, split):
                # projection units for segment s (dma_ytr(s) must precede)
                units = []
                obs = {}

                def mk_ob(m):
                    obs[m] = work.tile([128, C], F32, tag="ob", bufs=8,
                                       name=f"ob{s}_{m}")
                    return obs[m]

                for m in range(4):
                    for vn in range(2):
                        b = pbanks[(m * 2 + vn) % len(pbanks)]
                        ob = mk_ob(m) if vn == 0 else obs[m]
                        if split:
                            for half in range(2):
                                units.append((COST_D // 2, lambda s=s, m=m, vn=vn,
                                              half=half, b=b, ob=ob: proj_half(
                                                  s, m, vn, half, b, ob, eng,
                                                  defer=True)))
                        else:
                            units.append((COST_D, lambda s=s, m=m, vn=vn, b=b,
                                          ob=ob: (proj_half(s, m, vn, 0, b, ob, eng),
                                                  proj_half(s, m, vn, 1, b, ob, eng))))
                return units

            # ---------- program ----------
            # prologue DMA order (DMA issue/transfer are serialized resources):
            # x^T(seg0) -> Wq -> Wk -> x^T(seg1) -> Wv -> Wproj
            dma_xT(0)
            for cc in range(8):
                nc.sync.dma_start(wa_sb[cc][:, 0:128],
                                  wa[cc * 128:(cc + 1) * 128, 0:128])
            for cc in range(8):
                nc.sync.dma_start(wa_sb[cc][:, 128:C],
                                  wa[cc * 128:(cc + 1) * 128, 128:C])
            for cc in range(8):
                nc.sync.dma_start(wa_sb[cc][:, C:2 * C],
                                  wa[cc * 128:(cc + 1) * 128, C:2 * C])
            dma_xT(1)
            for cc in range(8):
                nc.sync.dma_start(wa_sb[cc][:, 2 * C:3 * C],
                                  wa[cc * 128:(cc + 1) * 128, 2 * C:3 * C])
            for cc in range(8):
                nc.sync.dma_start(wp_sb[cc][:], wp[cc * 128:(cc + 1) * 128, :])
            for u in range(16):
                qk_unit(0, u, bu[u % 2])
            for u in range(8):
                v_unit(0, u, bu[u % 2])

            for s in range(NSEG):
                if s + 2 < NSEG:
                    dma_xT(s + 2)
                if s + 1 < NSEG:
                    fillers = [(COST_B, lambda b, s=s, u=u: qk_unit(s + 1, u, b))
                               for u in range(16)]
                    fillers += [(COST_B, lambda b, s=s, u=u: v_unit(s + 1, u, b))
                                for u in range(8)]
                    dpaced = False
                else:
                    fillers = [(c, lambda b, f=f: f())
                               for c, f in d_units(s - 1, [bu[0], bu[1]],
                                                   nc.vector.tensor_copy, True)]
                    dpaced = True  # one unit per slot: 16 units / 16 slots
                nb = 0
                for h in range(H):
                    head_s(s, h)
                    if h > 0:
                        head_av(s, h - 1)
                    got = big = n = 0
                    while fillers and got < SLOT_FILL and big < 1 \
                            and not (dpaced and n >= 1):
                        c, f = fillers.pop(0)
                        f(bu[nb % 2])
                        nb += 1
                        got += c
                        big += c > 500
                        n += 1
                if fillers:
                    _, f = fillers.pop(0)
                    f(bu[nb % 2])
                    nb += 1
                head_av(s, H - 1)
                dma_ytr(s)  # issue early: latency hides under the drain
                for dd in deferred_dmas:
                    dd()
                deferred_dmas.clear()
                for _, f in fillers:
                    f(bu[nb % 2])
                    nb += 1
                if s + 1 < NSEG:
                    if s != NSEG - 2:
                        for _, f in d_units(s, [sbank(0), sbank(1), sbank(2),
                                               sbank(3)], nc.scalar.copy, False):
                            f()
                else:
                    for _, f in d_units(s, [sbank(0), sbank(1), sbank(2),
                                           sbank(3)], nc.scalar.copy, False):
                        f()

    _split_multi_waits(nc)
    return nc


_NC = None


def kernel(x, w_attn, w_proj, split_sections):
    global _NC
    if _NC is None:
        _NC = _build()
    x = np.asarray(x, dtype=np.float32).astype(ml_dtypes.bfloat16)
    w_attn = np.asarray(w_attn, dtype=np.float32).astype(ml_dtypes.bfloat16)
    w_proj = np.asarray(w_proj, dtype=np.float32).astype(ml_dtypes.bfloat16)
    in_maps = [
        {"x_sh": np.ascontiguousarray(x[i * TOK:(i + 1) * TOK]),
         "w_attn": w_attn, "w_proj": w_proj}
        for i in range(NCORES)
    ]
    res = run_bass_kernel_spmd(_NC, in_maps, core_ids=list(range(NCORES)))
    return np.concatenate([res.results[i]["out"] for i in range(NCORES)], axis=0)


if __name__ == "__main__":
    rng = np.random.default_rng(0)
    x = rng.standard_normal((T, C), dtype=np.float32)
    wa = (rng.standard_normal((C, 3 * C), dtype=np.float32) / np.sqrt(C)).astype(np.float32)
    wpj = (rng.standard_normal((C, C), dtype=np.float32) / np.sqrt(C)).astype(np.float32)
    y = kernel(x, wa, wpj, np.arange(1, 32) * 512)
    print("out", y.shape, y.dtype, np.abs(y).mean())
